# revision 2
# baseline (speedup 1.0000x reference)
"""Patch-entropy (histogram binning) Trainium2 Bass kernel, v3.

Input x:[64,3,512,512] f32 -> out:[64,32,32] f32; 8 NeuronCores data-parallel
(8 images per core).

Per-core pipeline, 2 groups of 4 images; partition = (img4, patchrow32):
  1. DMA rows as channel tiles [128, 512] f32 (256KB each, 96 total).
  2. DVE gray (bit-identical to reference fp32 chain):
     ts t0=W0*R (2x), stt t1=W1*G+t0 (1x), stt y=W2*B+t1 with the output AP
     shuffling the free dim to (pc32, r16, c16) so each partition holds 32
     whole patches contiguously.  W_c = 32*w_c (exact *2^5).
  3. DVE bitwise-AND clears low 13 mantissa bits (fp16-exact truncation;
     [trunc(y) >= b] == [y >= b] for integer b), then copy-cast f32->fp16.
  4. Histogram staircase S_b = #{y >= b} per patch, split across engines
     with per-engine stair tiles (no cross-engine WAW serialization):
     - DVE bins: tensor_scalar is_ge fp16 2x + fused accum (~163ns/op)
     - ScalarE bins: activation Sign(y - (b-delta)) f32 + fused accum
       (delta lies strictly between fp16-trunc grid points -> no Sign(0)
       ties); S = acc/2 + 128.
  5. counts by staircase diffs (stitching the two stair tiles at the bin-range
     boundaries); ScalarE Ln; per-patch A = sum_b c_b*ln(c_b/256+eps).
  6. Per-image min/max: free-dim reduce + masked pack (BIG=8192 keeps A's
     precision) + gpsimd partition_all_reduce; norm = (Amax-A)*(-1/den),
     den = Amax-Amin+256eps -- matches reference normalization exactly.
  7. One contiguous [128, 64=(g2,pc32)] DMA out; host reorders to [8,32,32].
"""
import numpy as np
from contextlib import ExitStack

import concourse.bass as bass
import concourse.bacc as bacc
import concourse.tile as tile
import concourse.mybir as mybir
from concourse import bass_isa

F32 = mybir.dt.float32
F16 = mybir.dt.float16
BF16 = mybir.dt.bfloat16
I32 = mybir.dt.int32
AO = mybir.AluOpType
AF = mybir.ActivationFunctionType

N_CORES = 8
IMG_PER_CORE = 8
C, H, W = 3, 512, 512
PS = 16
NB = 32
GRAY_W = (0.299, 0.587, 0.114)
EPS = 1e-8
MASK = 0xFFFFE000  # keep 10 mantissa bits -> value fp16-representable
BIG = 8192.0  # |A| < 2048; A+BIG keeps ~1e-3 abs precision, exact unmask

SCE_LO, SCE_HI = 12, 13  # E1: only 2 ScalarE bins
SCE_BINS = list(range(SCE_LO, SCE_HI + 1))
DVE_BINS = [b for b in range(1, NB) if b not in SCE_BINS]


def _sign_bias(b):
    """-(b - delta): delta strictly between fp16-trunc grid points below b."""
    e = int(np.floor(np.log2(b - 0.25)))  # exponent of values just below b
    delta = 2.0 ** (e - 12)
    return -(float(b) - delta)


def consts_np():
    a = np.zeros((128, 128), np.float32)
    # cols 32..63: sign bias for bin b at col 32+b
    for b in range(1, NB):
        a[:, 32 + b] = _sign_bias(b)
    # cols 64..67: image-extraction masks: partition p=(i4,pr32), i(p)=p>>5
    p = np.arange(128)
    i_of_p = p >> 5
    for m in range(4):
        a[:, 64 + m] = (i_of_p == m).astype(np.float32)
    return a


def _build_body(ctx, tc, x, consts, out, dbg_y=None, dbg_st=None):
    nc = tc.nc

    const_pool = ctx.enter_context(tc.tile_pool(name="const", bufs=1))
    ch_pool = ctx.enter_context(tc.tile_pool(name="ch", bufs=3))
    q_pool = ctx.enter_context(tc.tile_pool(name="q", bufs=2))
    qh_pool = ctx.enter_context(tc.tile_pool(name="qh", bufs=1))
    sp = ctx.enter_context(tc.tile_pool(name="sp", bufs=1))
    dd_pool = ctx.enter_context(tc.tile_pool(name="dd", bufs=4))
    ds_pool = ctx.enter_context(tc.tile_pool(name="ds", bufs=4))

    cmat = const_pool.tile([128, 128], F32, tag="cmat")
    nc.sync.dma_start(out=cmat[:], in_=consts[:])
    mask4 = cmat[:, 64:68]

    st = sp.tile([128, 31 * 64], F32, tag="stair")
    stv = st.rearrange("p (b t) -> p b t", t=64)
    cmp_pool = ctx.enter_context(tc.tile_pool(name="cmp", bufs=2))
    w_pool = ctx.enter_context(tc.tile_pool(name="w", bufs=2))

    W0 = float(np.float32(32.0) * np.float32(GRAY_W[0]))
    W1 = float(np.float32(32.0) * np.float32(GRAY_W[1]))
    W2 = float(np.float32(32.0) * np.float32(GRAY_W[2]))

    x_r = x.rearrange("b c (pr r) w -> b c pr r w", r=PS)

    for g in range(2):
        q = q_pool.tile([128, 8192], F32, tag="q")
        qv = q.rearrange("p (h1 h2 h3 pc r0 c) -> p h1 h2 h3 pc r0 c",
                         h1=2, h2=2, h3=2, pc=32, r0=2)
        for r in range(PS):
            xts = []
            for c in range(3):
                xt = ch_pool.tile([128, 512], F32, tag=f"xt{c}")
                nc.sync.dma_start(out=xt[:], in_=x_r[4 * g : 4 * g + 4, c, :, r, :])
                xts.append(xt)
            t0 = ch_pool.tile([128, 512], F32, tag="t0")
            nc.vector.tensor_scalar(t0[:], xts[0][:], W0, None,
                                    op0=AO.mult, op1=AO.bypass)
            t1 = ch_pool.tile([128, 512], F32, tag="t1")
            nc.vector.scalar_tensor_tensor(t1[:], xts[1][:], W1, t0[:],
                                           op0=AO.mult, op1=AO.add)
            nc.vector.scalar_tensor_tensor(
                qv[:, (r >> 3) & 1, (r >> 2) & 1, (r >> 1) & 1, :, r & 1, :],
                xts[2][:].rearrange("p (pc c) -> p pc c", c=PS),
                W2,
                t1[:].rearrange("p (pc c) -> p pc c", c=PS),
                op0=AO.mult, op1=AO.add,
            )

        if dbg_y is not None and g == 0:
            nc.sync.dma_start(out=dbg_y, in_=q[:])
        nc.vector.tensor_scalar(
            q[:].bitcast(I32), q[:].bitcast(I32), MASK, None,
            op0=AO.bitwise_and, op1=AO.bypass,
        )
        qh = qh_pool.tile([128, 8192], F16, tag="qh")
        nc.vector.tensor_copy(qh[:], q[:])

        for b in range(1, NB):
            cmp = cmp_pool.tile([128, 8192], F16, tag="cmp")
            nc.vector.tensor_scalar(
                cmp[:], qh[:], float(b), None, op0=AO.is_ge, op1=AO.bypass,
            )
            w1 = w_pool.tile([128, 4096], F16, tag="w1")
            nc.vector.tensor_tensor(w1[:], cmp[:, 0:4096], cmp[:, 4096:8192],
                                    op=AO.add)
            w2 = w_pool.tile([128, 2048], F16, tag="w2")
            nc.vector.tensor_tensor(w2[:], w1[:, 0:2048], w1[:, 2048:4096],
                                    op=AO.add)
            w3 = w_pool.tile([128, 1024], F16, tag="w3")
            nc.vector.tensor_tensor(w3[:], w2[:, 0:1024], w2[:, 1024:2048],
                                    op=AO.add)
            nc.vector.tensor_reduce(
                stv[:, b - 1, g * 32 : (g + 1) * 32],
                w3.rearrange("p (pc n) -> p pc n", pc=32),
                axis=mybir.AxisListType.X, op=AO.add,
            )

    # counts plane-major: c_b = S_b - S_{b+1}; S_0 = 256, S_32 = 0.
    counts = sp.tile([128, NB * 64], F32, tag="counts")
    cpv = counts.rearrange("p (b t) -> p b t", t=64)
    nc.vector.tensor_scalar(cpv[:, 0], stv[:, 0], -1.0, 256.0,
                            op0=AO.mult, op1=AO.add)
    nc.vector.tensor_sub(cpv[:, 1:31], stv[:, 0:30], stv[:, 1:31])
    nc.vector.tensor_copy(cpv[:, 31], stv[:, 30])

    pe = sp.tile([128, 64 * NB], F32, tag="pe")
    nc.vector.tensor_scalar(pe[:], counts[:], 1.0 / 256.0, EPS,
                            op0=AO.mult, op1=AO.add)
    lnpe = pe  # Ln in place (elementwise 1:1)
    nc.scalar.activation(lnpe[:], pe[:], AF.Ln)

    prod = lnpe  # counts * ln(pe), in place over lnpe
    nc.vector.tensor_mul(prod[:], counts[:], lnpe[:])
    pv = prod.rearrange("p (h n) -> p h n", h=2)
    nc.vector.tensor_add(pv[:, 0], pv[:, 0], pv[:, 1])  # 32 -> 16 planes
    p2 = prod.rearrange("p (h n) -> p h n", h=4)
    nc.vector.tensor_add(p2[:, 0], p2[:, 0], p2[:, 1])  # 16 -> 8
    p3 = prod.rearrange("p (h n) -> p h n", h=8)
    nc.vector.tensor_add(p3[:, 0], p3[:, 0], p3[:, 1])  # 8 -> 4
    p4 = prod.rearrange("p (h n) -> p h n", h=16)
    nc.vector.tensor_add(p4[:, 0], p4[:, 0], p4[:, 1])  # 4 -> 2
    A = sp.tile([128, 64], F32, tag="A")
    nc.vector.tensor_add(A[:], p4[:, 0, 0:64], p4[:, 0, 64:128])

    # per-(partition, g) max/min over 32 patches
    Av = A.rearrange("p (g n) -> p g n", n=32)
    amax_g = sp.tile([128, 2], F32, tag="amax_g")
    nc.vector.tensor_reduce(amax_g[:], Av, axis=mybir.AxisListType.X, op=AO.max)
    amin_g = sp.tile([128, 2], F32, tag="amin_g")
    nc.vector.tensor_reduce(amin_g[:], Av, axis=mybir.AxisListType.X, op=AO.min)

    # masked pack [128, 8]: col m=(g*4+i); -BIG elsewhere
    tmax = sp.tile([128, 2], F32, tag="tmax")
    nc.vector.tensor_scalar(tmax[:], amax_g[:], 1.0, BIG, op0=AO.mult, op1=AO.add)
    tmin = sp.tile([128, 2], F32, tag="tmin")
    nc.vector.tensor_scalar(tmin[:], amin_g[:], -1.0, BIG, op0=AO.mult, op1=AO.add)
    m8x = sp.tile([128, 8], F32, tag="m8x")
    m8n = sp.tile([128, 8], F32, tag="m8n")
    for gg in range(2):
        nc.vector.tensor_scalar(
            m8x[:, 4 * gg : 4 * gg + 4], mask4,
            tmax[:, gg : gg + 1], BIG, op0=AO.mult, op1=AO.subtract,
        )
        nc.vector.tensor_scalar(
            m8n[:, 4 * gg : 4 * gg + 4], mask4,
            tmin[:, gg : gg + 1], BIG, op0=AO.mult, op1=AO.subtract,
        )
    m8xr = sp.tile([128, 8], F32, tag="m8xr")
    nc.gpsimd.partition_all_reduce(m8xr[:], m8x[:], channels=128,
                                   reduce_op=bass_isa.ReduceOp.max)
    m8nr = sp.tile([128, 8], F32, tag="m8nr")
    nc.gpsimd.partition_all_reduce(m8nr[:], m8n[:], channels=128,
                                   reduce_op=bass_isa.ReduceOp.max)

    # extract per-partition per-g values via mask-sum
    mask8v = sp.tile([128, 8], F32, tag="mask8v")
    nc.vector.tensor_copy(mask8v[:, 0:4], mask4)
    nc.vector.tensor_copy(mask8v[:, 4:8], mask4)
    mx = sp.tile([128, 8], F32, tag="mx")
    nc.vector.tensor_mul(mx[:], m8xr[:], mask8v[:])
    amax_pp = sp.tile([128, 2], F32, tag="amax_pp")
    nc.vector.tensor_reduce(amax_pp[:], mx.rearrange("p (g n) -> p g n", n=4),
                            axis=mybir.AxisListType.X, op=AO.add)
    mn = sp.tile([128, 8], F32, tag="mn")
    nc.vector.tensor_mul(mn[:], m8nr[:], mask8v[:])
    negmin_pp = sp.tile([128, 2], F32, tag="negmin_pp")
    nc.vector.tensor_reduce(negmin_pp[:], mn.rearrange("p (g n) -> p g n", n=4),
                            axis=mybir.AxisListType.X, op=AO.add)

    # rneg = -1/(amax - amin + 256*EPS)
    dneg = sp.tile([128, 2], F32, tag="dneg")
    nc.vector.scalar_tensor_tensor(
        dneg[:], amax_pp[:], -1.0, negmin_pp[:], op0=AO.mult, op1=AO.subtract
    )
    nc.vector.tensor_scalar(dneg[:], dneg[:], float(256.0 * EPS), None,
                            op0=AO.subtract, op1=AO.bypass)
    rneg = sp.tile([128, 2], F32, tag="rneg")
    nc.vector.reciprocal(rneg[:], dneg[:])

    norm = sp.tile([128, 64], F32, tag="norm")
    for gg in range(2):
        nc.vector.tensor_scalar(
            norm[:, 32 * gg : 32 * (gg + 1)], A[:, 32 * gg : 32 * (gg + 1)],
            amax_pp[:, gg : gg + 1], rneg[:, gg : gg + 1],
            op0=AO.subtract, op1=AO.mult,
        )
    nc.sync.dma_start(out=out[:], in_=norm[:])


def build_program():
    nc = bacc.Bacc(target_bir_lowering=True)
    x = nc.declare_dram_parameter("x", [IMG_PER_CORE, C, H, W], F32, isOutput=False)
    consts = nc.declare_dram_parameter("consts", [128, 128], F32, isOutput=False)
    out = nc.declare_dram_parameter("out", [128, 64], F32, isOutput=True)
    with tile.TileContext(nc) as tc:
        with ExitStack() as ctx:
            _build_body(ctx, tc, x[:], consts[:], out[:])
    return nc


_CACHED = {}


def _get_program():
    if "nc" not in _CACHED:
        nc = build_program()
        nc.finalize()
        _CACHED["nc"] = nc
    return _CACHED["nc"]


def unshuffle(raw):
    """raw [128, 64] -> [8, 32, 32]; partition p=(i4,pr32), free f=(g2,pc32)."""
    v = raw.reshape(4, 32, 2, 32)  # i pr g pc
    return np.ascontiguousarray(v.transpose(2, 0, 1, 3)).reshape(8, 32, 32)


def kernel(x, patch_size, num_bins):
    assert int(patch_size) == PS and int(num_bins) == NB
    x = np.asarray(x, dtype=np.float32)
    B = x.shape[0]
    assert x.shape == (B, C, H, W) and B % N_CORES == 0
    per = B // N_CORES
    assert per == IMG_PER_CORE
    nc = _get_program()

    cns = consts_np()
    in_maps = [
        {"x": x[i * per : (i + 1) * per], "consts": cns} for i in range(N_CORES)
    ]
    from concourse.bass_utils import run_bass_kernel_spmd

    res = run_bass_kernel_spmd(nc, in_maps, list(range(N_CORES)), trace=False)
    return np.concatenate(
        [unshuffle(res.results[i]["out"]) for i in range(N_CORES)], axis=0
    )
